# revision 2
# baseline (speedup 1.0000x reference)
"""Neighbourhood attention block (7x7 clamped window) on 8 Trainium2 cores, v2.

Sharding: (batch, head-pair) tensor parallel. Core c handles batch b = c//4
and heads (2*(c%4), 2*(c%4)+1). Each core computes q/k/v projections for its
two heads, neighbourhood attention, and a partial output projection in bf16;
host sums the 4 bf16 partials per batch in f32.

v2 layout: one scores tile per key chunk c (2 image rows = 128 keys), queries
= exactly the rows that see the chunk (nq = 320..704 cols), scoresT [key, q].
Probs = exp(scale*scores) * mask (bf16, one exp per chunk-pair). PV is
transposed: per query-chunk J (2 image rows = 128 queries), probs slices
[128 k, <=128 q] are the matmul stationary and V [128 k, 65] the moving
operand, accumulating ao [128 q, 130] in PSUM (cols 64/129 = softmax
denominators via ones columns in V). The reciprocal is applied on the
PSUM->SBUF copy (per-partition scalar), ao is transposed on the PE, and the
output projection consumes aoT chunks as stationary against a resident
wo [128, 512] moving operand, yielding y [128 tok, 512] per chunk.
"""
import os
import numpy as np
import ml_dtypes
from contextlib import ExitStack

_PHASES = os.environ.get("KERNEL_PHASES", "123")  # debug bisect knob

import concourse.bass as bass
import concourse.bacc as bacc
import concourse.tile as tile
import concourse.mybir as mybir
from concourse.bass_utils import run_bass_kernel_spmd
from concourse.masks import make_identity

F32 = mybir.dt.float32
BF16 = mybir.dt.bfloat16

B, H, W, D = 2, 64, 64, 512
DH, NH = 64, 8
S = H * W              # 4096 tokens per batch
KER = 7
SCALE = DH ** -0.5     # 0.125
NCORES = 8

# ---------------------------------------------------------------- geometry

def _sh(r):            # clamped window start (rows); same formula for cols
    return min(max(r - KER // 2, 0), H - KER)


def _chunks_of_row(r):  # key chunks (2 rows each) seen by query row r
    s = _sh(r)
    return list(range(s // 2, (s + KER + 1) // 2))


def _build_plan():
    # one scores tile per key chunk: queries = all rows seeing the chunk
    tiles = []          # per c: dict(c, lo, hi, nq)
    for c in range(32):
        rows = [r for r in range(H) if c in _chunks_of_row(r)]
        assert rows == list(range(rows[0], rows[-1] + 1))
        tiles.append(dict(c=c, lo=rows[0], hi=rows[-1],
                          nq=(rows[-1] - rows[0] + 1) * 64))
    assert sum(t["nq"] for t in tiles) == 64 * sum(
        len(_chunks_of_row(r)) for r in range(H))

    # exp/mask groups: greedy pairing of consecutive chunks, width <= 1024
    pairs = []          # dict(cs, width, mask_id)
    c = 0
    while c < 32:
        if c + 1 < 32 and tiles[c]["nq"] + tiles[c + 1]["nq"] <= 1024:
            pairs.append(dict(cs=[c, c + 1],
                              width=tiles[c]["nq"] + tiles[c + 1]["nq"]))
            c += 2
        else:
            pairs.append(dict(cs=[c], width=tiles[c]["nq"]))
            c += 1
    pair_of = {}        # chunk -> pair index
    for pi, p in enumerate(pairs):
        for cc in p["cs"]:
            pair_of[cc] = pi

    # PV plan per query-chunk J (rows 2J, 2J+1)
    pvplan = []
    for J in range(32):
        segs = []       # (c, tile_off_cols, out_row_off, nrows)
        for cc in sorted(set(_chunks_of_row(2 * J)) | set(_chunks_of_row(2 * J + 1))):
            rp = [r for r in (2 * J, 2 * J + 1) if cc in _chunks_of_row(r)]
            assert rp == list(range(rp[0], rp[-1] + 1))
            t = tiles[cc]
            segs.append((cc, (rp[0] - t["lo"]) * 64, (rp[0] - 2 * J) * 64,
                         len(rp)))
        # order: a full (2-row) seg opens the accumulation group, half segs
        # in the middle, and a full seg closes it (stop must cover all rows)
        full = [g for g in segs if g[3] == 2]
        half = [g for g in segs if g[3] == 1]
        assert len(full) >= 2, (J, segs)
        segs = full[:1] + half + full[1:]
        pvplan.append(segs)

    # sanity: every (query row, chunk) incidence consumed exactly once
    seen = set()
    for J, segs in enumerate(pvplan):
        for cc, toff, ooff, nr in segs:
            for k in range(nr):
                key = (2 * J + ooff // 64 + k, cc)
                assert key not in seen, key
                seen.add(key)
    for r in range(H):
        for cc in _chunks_of_row(r):
            assert (r, cc) in seen, (r, cc)

    # masks per pair (0/1), deduped. mask[k, q] over the pair's concat q-cols
    starts = np.minimum(np.maximum(np.arange(H) - KER // 2, 0), H - KER)
    valid = (np.arange(H)[None, :] >= starts[:, None]) & \
            (np.arange(H)[None, :] < starts[:, None] + KER)   # [q pos, k pos]

    def chunk_mask(cc):
        t = tiles[cc]
        ktok = cc * 128 + np.arange(128)
        qtok = t["lo"] * 64 + np.arange(t["nq"])
        return (valid[qtok[None, :] // 64, ktok[:, None] // 64]
                & valid[qtok[None, :] % 64, ktok[:, None] % 64])

    mask_list, mask_ids = [], {}
    for p in pairs:
        m = np.zeros((128, 1024), np.float32)
        off = 0
        for cc in p["cs"]:
            w = tiles[cc]["nq"]
            m[:, off:off + w] = chunk_mask(cc)
            off += w
        key = m.tobytes()
        if key not in mask_ids:
            mask_ids[key] = len(mask_list)
            mask_list.append(m)
        p["mask_id"] = mask_ids[key]
    return tiles, pairs, pair_of, pvplan, np.stack(mask_list)


TILES, PAIRS, PAIR_OF, PVPLAN, MASKS = _build_plan()
NMASK = len(MASKS)

# ---------------------------------------------------------------- device

_NC_CACHE = {}
TRACE = False          # set True (e.g. from test.py) to capture an NTFF profile
LAST_RESULTS = None    # BassKernelResults of the most recent kernel() call


def _build_module():
    nc = bacc.Bacc("TRN2", target_bir_lowering=False, debug=False,
                   num_devices=NCORES)
    xT_d = nc.dram_tensor("xT", [128, 4, S], BF16, kind="ExternalInput")
    wqk_d = nc.dram_tensor("wqk", [128, 8, 128], BF16, kind="ExternalInput")
    wv_d = nc.dram_tensor("wv", [128, 4, 128], BF16, kind="ExternalInput")
    wo_d = nc.dram_tensor("wo", [128, 512], BF16, kind="ExternalInput")
    mk_d = nc.dram_tensor("masks", [128, NMASK, 1024], BF16, kind="ExternalInput")
    y_d = nc.dram_tensor("y", [8, 128, 4, 512], BF16, kind="ExternalOutput")

    with tile.TileContext(nc) as tc, ExitStack() as ctx:
        const = ctx.enter_context(tc.tile_pool(name="const", bufs=1))
        # SP queue: q/k weights, then x token chunks interleaved with the
        # remaining small inputs. Pool queue: masks (first half first).
        xT_t = const.tile([128, 4, S], BF16, tag="xT")
        nc.sync.dma_start(out=xT_t[:, :, 0:256], in_=xT_d[:, :, 0:256])
        wqk_t = const.tile([128, 8, 128], BF16, tag="wqk")
        nc.sync.dma_start(out=wqk_t[:], in_=wqk_d[:, :, :])
        wq_t, wk_t = wqk_t[:, 0:4, :], wqk_t[:, 4:8, :]
        nc.sync.dma_start(out=xT_t[:, :, 256:512], in_=xT_d[:, :, 256:512])
        mk_t = const.tile([128, NMASK, 1024], BF16, tag="mk")
        mhalf = min(4, NMASK)
        nc.gpsimd.dma_start(out=mk_t[:, 0:mhalf, :], in_=mk_d[:, 0:mhalf, :])
        if NMASK > mhalf:
            nc.gpsimd.dma_start(out=mk_t[:, mhalf:, :], in_=mk_d[:, mhalf:, :])
        wv_t = const.tile([128, 4, 128], BF16, tag="wv")
        wo_t = const.tile([128, 512], BF16, tag="wo")
        for i in range(1, 8):
            nc.sync.dma_start(out=xT_t[:, :, i * 512:(i + 1) * 512],
                              in_=xT_d[:, :, i * 512:(i + 1) * 512])
            if i == 1:
                nc.sync.dma_start(out=wv_t[:], in_=wv_d[:, :, :])
            elif i == 2:
                nc.sync.dma_start(out=wo_t[:], in_=wo_d[:, :])

        qkT = const.tile([128, 2, S], BF16, tag="qkT")  # [:,0,:]=q, [:,1,:]=k
        # V: [tok_in_chunk, chunk, 130]: cols 0:64 u0-e, 64 ones, 65:129 u1-e, 129 ones
        V = const.tile([128, 32, 130], BF16, tag="V")
        nc.gpsimd.memset(V[:, :, 64:130:65], 1.0)   # only the ones columns
        identb = const.tile([128, 128], BF16, tag="identb")
        identf = const.tile([128, 128], F32, tag="identf")
        make_identity(nc, identf[:])
        nc.vector.tensor_copy(identb[:], identf[:])

        with tc.tile_pool(name="sps", bufs=2, space="PSUM") as sps, \
             tc.tile_pool(name="pvs", bufs=2, space="PSUM") as pvs, \
             tc.tile_pool(name="ops", bufs=2, space="PSUM") as ops, \
             tc.tile_pool(name="prp", bufs=14) as prp, \
             tc.tile_pool(name="aux", bufs=4) as aux, \
             tc.tile_pool(name="yp", bufs=2) as yp:

            # Lazy emitters so Python emission order matches consumption
            # order (pool slots rotate FIFO in emission order). GPSIMD/Pool
            # cannot touch PSUM, so all PSUM reads sit on DVE/Act.
            done_qk = set()     # token blocks with q/k projections emitted
            done_v = set()      # v-proj chunk pairs emitted
            probs = [{}, {}]    # unit -> chunk -> AP view [128, nq]

            def need_qk(nb):    # q+k -> qkT [dim, tok], 512-token block nb
                if nb in done_qk or "1" not in _PHASES:
                    return
                done_qk.add(nb)
                subs = ((0, 256), (256, 512)) if nb == 0 else ((0, 512),)
                for qk, w_t in ((0, wq_t), (1, wk_t)):
                    acc = ops.tile([128, 512], F32, tag="yacc")
                    for a, b in subs:
                        for dc in range(4):
                            nc.tensor.matmul(
                                acc[:, a:b], w_t[:, dc, :],
                                xT_t[:, dc, nb * 512 + a:nb * 512 + b],
                                start=(dc == 0), stop=(dc == 3))
                    if qk == 0:
                        nc.scalar.activation(
                            qkT[:, 0, nb * 512:(nb + 1) * 512], acc[:],
                            mybir.ActivationFunctionType.Copy)
                    else:
                        nc.vector.tensor_copy(
                            qkT[:, 1, nb * 512:(nb + 1) * 512], acc[:])

            def need_v(ci):     # v direct to [tok, dim] for chunks {ci, ci+1}
                ci -= ci % 2
                if ci in done_v or "1" not in _PHASES:
                    return
                done_v.add(ci)
                acc = ops.tile([128, 512], F32, tag="yacc")
                for h in range(2):
                    for dc in range(4):
                        nc.tensor.matmul(
                            acc[:, h * 128:h * 128 + 128],
                            xT_t[:, dc, (ci + h) * 128:(ci + h + 1) * 128],
                            wv_t[:, dc, :],
                            start=(dc == 0), stop=(dc == 3))
                # [128 tok, 2, 128 vdim] -> V cols {0:64, 65:129} of 2 chunks
                nc.vector.tensor_copy(
                    V[:, ci:ci + 2, :].rearrange("p c (t w) -> p c t w", t=2)[:, :, :, 0:64],
                    acc[:, 0:256].rearrange("p (c t w) -> p c t w", c=2, t=2))

            NPAIRS = len(PAIRS)

            def emit_pair(u, pi):
                p = PAIRS[pi]
                ue = slice(u * 64, u * 64 + 64)
                wdt = p["width"]
                hi_tok = (TILES[p["cs"][-1]]["hi"] + 1) * 64
                for nb in range((hi_tok + 511) // 512):
                    need_qk(nb)
                sc = sps.tile([128, 1024], F32, tag="sc")
                off = 0
                for cc in p["cs"]:
                    t = TILES[cc]
                    q0 = t["lo"] * 64
                    pos = 0
                    while pos < t["nq"]:
                        # PSUM: each matmul must stay inside one 512-col bank
                        end = min(t["nq"], pos + 512,
                                  ((off + pos) // 512 + 1) * 512 - off)
                        nc.tensor.matmul(sc[:, off + pos:off + end],
                                         qkT[ue, 1, cc * 128:(cc + 1) * 128],
                                         qkT[ue, 0, q0 + pos:q0 + end],
                                         start=True, stop=True)
                        pos = end
                    off += t["nq"]
                pr = prp.tile([128, 1024], BF16, tag="pr")
                nc.scalar.activation(pr[:, :wdt], sc[:, :wdt],
                                     mybir.ActivationFunctionType.Exp,
                                     scale=SCALE)
                nc.gpsimd.tensor_mul(pr[:, :wdt], pr[:, :wdt],
                                     mk_t[:, p["mask_id"], :wdt])
                off = 0
                for cc in p["cs"]:
                    probs[u][cc] = pr[:, off:off + TILES[cc]["nq"]]
                    off += TILES[cc]["nq"]

            def need_probs(J):  # probs + V for all chunks query-chunk J uses
                if J >= 32:
                    return
                for u in (0, 1):
                    for cc, _t, _o, _n in PVPLAN[J]:
                        if cc not in probs[u]:
                            emit_pair(u, PAIR_OF[cc])
                        need_v(cc)

            y4 = None
            state = {}          # Bk -> (pv, ao) awaiting their tail pass

            def emit_pv(Bk):    # PV + normalize for 2-J block Bk  (PE, DVE)
                # one PSUM bank per block: ao accumulation in cols 0:260
                # (J-even u0/u1, J-odd u0/u1, each 65 wide incl denominator),
                # bf16 transpose targets carved at f32 cols 320:448
                pv = pvs.tile([128, 512], F32, tag="pv")
                for J in (2 * Bk, 2 * Bk + 1):
                    base = 130 * (J % 2)
                    for u in (0, 1):
                        uv = slice(base + u * 65, base + u * 65 + 65)
                        segs = PVPLAN[J]
                        for si, (cc, toff, ooff, nr) in enumerate(segs):
                            nc.tensor.matmul(pv[ooff:ooff + nr * 64, uv],
                                             probs[u][cc][:, toff:toff + nr * 64],
                                             V[:, cc, u * 65:u * 65 + 65],
                                             start=(si == 0),
                                             stop=(si == len(segs) - 1))
                recip = aux.tile([128, 4], F32, tag="recip")
                nc.vector.reciprocal(recip[:], pv[:, 64:260:65])
                # normalize all 4 (J, unit) panels in one broadcast multiply
                ao = aux.tile([128, 4, 64], BF16, tag="ao")
                nc.vector.tensor_tensor(
                    ao[:],
                    pv[:, 0:260].rearrange("p (t w) -> p t w", t=4)[:, :, 0:64],
                    recip[:].unsqueeze(2).broadcast_to([128, 4, 64]),
                    mybir.AluOpType.mult)
                state[Bk] = (pv, ao)

            def emit_tail_a(Bk):    # transposes [PE] + aoT copy [DVE]
                pv, ao = state.pop(Bk)
                tp = pv[:, 320:448].bitcast(BF16)      # [128, 2, 128] bf16
                nc.tensor.transpose(tp[:, 0:128], ao[:, 0:2, :], identb[:])
                nc.tensor.transpose(tp[:, 128:256], ao[:, 2:4, :], identb[:])
                aoT = aux.tile([128, 256], BF16, tag="aoT")
                nc.vector.tensor_copy(aoT[:], tp)
                state[("T", Bk)] = aoT

            def emit_tail_b(Bk):    # out-proj [PE] + y copies + DMA
                nonlocal y4
                aoT = state.pop(("T", Bk))
                if Bk % 2 == 0:
                    y4 = yp.tile([128, 4, 512], BF16, tag="y4")
                for J in (2 * Bk, 2 * Bk + 1):
                    yac = ops.tile([128, 512], F32, tag="yacc")
                    nc.tensor.matmul(yac[:], aoT[:, (J % 2) * 128:(J % 2) * 128 + 128],
                                     wo_t[:], start=True, stop=True)
                    if Bk >= 12 and J % 2 == 0:
                        # endgame: alternate engines so the drain parallelizes
                        nc.scalar.activation(y4[:, J % 4, :], yac[:],
                                             mybir.ActivationFunctionType.Copy)
                    else:
                        nc.vector.tensor_copy(y4[:, J % 4, :], yac[:])
                    if Bk == 15:
                        nc.sync.dma_start(out=y_d[7][:, J % 4:J % 4 + 1, :],
                                          in_=y4[:, J % 4:J % 4 + 1, :])
                if Bk < 15:
                    half = (Bk % 2) * 2
                    nc.sync.dma_start(out=y_d[Bk // 2][:, half:half + 2, :],
                                      in_=y4[:, half:half + 2, :])

            if "2" in _PHASES:
                for J in (0, 1, 2, 3):
                    need_probs(J)
                for Bk in range(16):
                    if Bk > 0 and "3" in _PHASES:
                        emit_tail_a(Bk - 1)
                    emit_pv(Bk)
                    if Bk > 0 and "3" in _PHASES:
                        emit_tail_b(Bk - 1)
                    for J in range(2 * Bk + 4, 2 * Bk + 8):
                        need_probs(J)       # two blocks of prefetch
                if "3" in _PHASES:
                    emit_tail_a(15)
                    emit_tail_b(15)
    nc.compile()
    return nc


def _get_module():
    if "nc" not in _NC_CACHE:
        _NC_CACHE["nc"] = _build_module()
    return _NC_CACHE["nc"]


# ---------------------------------------------------------------- host

def kernel(x, w_qkv, w_out):
    x = np.asarray(x, np.float32)
    w_qkv = np.asarray(w_qkv, np.float32)
    w_out = np.asarray(w_out, np.float32)
    nc = _get_module()

    bf = ml_dtypes.bfloat16
    masks_bf = np.ascontiguousarray(MASKS.transpose(1, 0, 2)).astype(bf)
    # xT [128, 4, S]: xT[p, c, t] = x[b].reshape(S, D).T[c*128+p, t]
    xT = [np.ascontiguousarray(
            x[b].reshape(S, D).T.reshape(4, 128, S).transpose(1, 0, 2)).astype(bf)
          for b in range(B)]
    w_outT = w_out.T    # [d, e]

    def wslice(base, f):  # [128, 4, 128]: w[p, c, o] = w_qkv[base+f+o? no]
        ws = w_qkv[base + f:base + f + 128]          # [128 out, 512 in]
        return np.ascontiguousarray(
            ws.T.reshape(4, 128, 128).transpose(1, 0, 2)).astype(bf)

    in_maps = []
    for c in range(NCORES):
        b, h0 = c // 4, 2 * (c % 4)
        f = h0 * 64
        in_maps.append({
            "xT": xT[b],
            "wqk": np.concatenate([wslice(0, f), wslice(512, f)], axis=1),
            "wv": wslice(1024, f),
            "wo": np.ascontiguousarray(w_outT[f:f + 128]).astype(bf),
            "masks": masks_bf,
        })
    res = run_bass_kernel_spmd(nc, in_maps, list(range(NCORES)), trace=TRACE)
    global LAST_RESULTS
    LAST_RESULTS = res
    y = np.zeros((B, S, D), np.float32)
    for c in range(NCORES):
        yc = np.asarray(res.results[c]["y"], dtype=np.float32)  # [8,128,4,512]
        y[c // 4] += yc.transpose(0, 2, 1, 3).reshape(S, D)
    return y.reshape(B, H, W, D)


# revision 4
# speedup vs baseline: 1.0252x; 1.0252x over previous
"""Neighbourhood attention block (7x7 clamped window) on 8 Trainium2 cores, v2.

Sharding: (batch, head-pair) tensor parallel. Core c handles batch b = c//4
and heads (2*(c%4), 2*(c%4)+1). Each core computes q/k/v projections for its
two heads, neighbourhood attention, and a partial output projection in bf16;
host sums the 4 bf16 partials per batch in f32.

v2 layout: one scores tile per key chunk c (2 image rows = 128 keys), queries
= exactly the rows that see the chunk (nq = 320..704 cols), scoresT [key, q].
Probs = exp(scale*scores) * mask (bf16, one exp per chunk-pair). PV is
transposed: per query-chunk J (2 image rows = 128 queries), probs slices
[128 k, <=128 q] are the matmul stationary and V [128 k, 65] the moving
operand, accumulating ao [128 q, 130] in PSUM (cols 64/129 = softmax
denominators via ones columns in V). The reciprocal is applied on the
PSUM->SBUF copy (per-partition scalar), ao is transposed on the PE, and the
output projection consumes aoT chunks as stationary against a resident
wo [128, 512] moving operand, yielding y [128 tok, 512] per chunk.
"""
import os
import numpy as np
import ml_dtypes
from contextlib import ExitStack

_PHASES = os.environ.get("KERNEL_PHASES", "123")  # debug bisect knob

import concourse.bass as bass
import concourse.bacc as bacc
import concourse.tile as tile
import concourse.mybir as mybir
from concourse.bass_utils import run_bass_kernel_spmd
from concourse.masks import make_identity

F32 = mybir.dt.float32
BF16 = mybir.dt.bfloat16

B, H, W, D = 2, 64, 64, 512
DH, NH = 64, 8
S = H * W              # 4096 tokens per batch
KER = 7
SCALE = DH ** -0.5     # 0.125
NCORES = 8

# ---------------------------------------------------------------- geometry

def _sh(r):            # clamped window start (rows); same formula for cols
    return min(max(r - KER // 2, 0), H - KER)


def _chunks_of_row(r):  # key chunks (2 rows each) seen by query row r
    s = _sh(r)
    return list(range(s // 2, (s + KER + 1) // 2))


def _build_plan():
    # one scores tile per key chunk: queries = all rows seeing the chunk
    tiles = []          # per c: dict(c, lo, hi, nq)
    for c in range(32):
        rows = [r for r in range(H) if c in _chunks_of_row(r)]
        assert rows == list(range(rows[0], rows[-1] + 1))
        tiles.append(dict(c=c, lo=rows[0], hi=rows[-1],
                          nq=(rows[-1] - rows[0] + 1) * 64))
    assert sum(t["nq"] for t in tiles) == 64 * sum(
        len(_chunks_of_row(r)) for r in range(H))

    # exp/mask groups: greedy pairing of consecutive chunks, width <= 1024
    pairs = []          # dict(cs, width, mask_id)
    c = 0
    while c < 32:
        if c + 1 < 32 and tiles[c]["nq"] + tiles[c + 1]["nq"] <= 1024:
            pairs.append(dict(cs=[c, c + 1],
                              width=tiles[c]["nq"] + tiles[c + 1]["nq"]))
            c += 2
        else:
            pairs.append(dict(cs=[c], width=tiles[c]["nq"]))
            c += 1
    pair_of = {}        # chunk -> pair index
    for pi, p in enumerate(pairs):
        for cc in p["cs"]:
            pair_of[cc] = pi

    # PV plan per query-chunk J (rows 2J, 2J+1)
    pvplan = []
    for J in range(32):
        segs = []       # (c, tile_off_cols, out_row_off, nrows)
        for cc in sorted(set(_chunks_of_row(2 * J)) | set(_chunks_of_row(2 * J + 1))):
            rp = [r for r in (2 * J, 2 * J + 1) if cc in _chunks_of_row(r)]
            assert rp == list(range(rp[0], rp[-1] + 1))
            t = tiles[cc]
            segs.append((cc, (rp[0] - t["lo"]) * 64, (rp[0] - 2 * J) * 64,
                         len(rp)))
        # order: a full (2-row) seg opens the accumulation group, half segs
        # in the middle, and a full seg closes it (stop must cover all rows)
        full = [g for g in segs if g[3] == 2]
        half = [g for g in segs if g[3] == 1]
        assert len(full) >= 2, (J, segs)
        segs = full[:1] + half + full[1:]
        pvplan.append(segs)

    # sanity: every (query row, chunk) incidence consumed exactly once
    seen = set()
    for J, segs in enumerate(pvplan):
        for cc, toff, ooff, nr in segs:
            for k in range(nr):
                key = (2 * J + ooff // 64 + k, cc)
                assert key not in seen, key
                seen.add(key)
    for r in range(H):
        for cc in _chunks_of_row(r):
            assert (r, cc) in seen, (r, cc)

    # masks per pair (0/1), deduped. mask[k, q] over the pair's concat q-cols
    starts = np.minimum(np.maximum(np.arange(H) - KER // 2, 0), H - KER)
    valid = (np.arange(H)[None, :] >= starts[:, None]) & \
            (np.arange(H)[None, :] < starts[:, None] + KER)   # [q pos, k pos]

    def chunk_mask(cc):
        t = tiles[cc]
        ktok = cc * 128 + np.arange(128)
        qtok = t["lo"] * 64 + np.arange(t["nq"])
        return (valid[qtok[None, :] // 64, ktok[:, None] // 64]
                & valid[qtok[None, :] % 64, ktok[:, None] % 64])

    mask_list, mask_ids = [], {}
    for p in pairs:
        m = np.zeros((128, 1024), np.float32)
        off = 0
        for cc in p["cs"]:
            w = tiles[cc]["nq"]
            m[:, off:off + w] = chunk_mask(cc)
            off += w
        key = m.tobytes()
        if key not in mask_ids:
            mask_ids[key] = len(mask_list)
            mask_list.append(m)
        p["mask_id"] = mask_ids[key]
    return tiles, pairs, pair_of, pvplan, np.stack(mask_list)


TILES, PAIRS, PAIR_OF, PVPLAN, MASKS = _build_plan()
NMASK = len(MASKS)

# ---------------------------------------------------------------- device

_NC_CACHE = {}
TRACE = False          # set True (e.g. from test.py) to capture an NTFF profile
LAST_RESULTS = None    # BassKernelResults of the most recent kernel() call


def _build_module():
    nc = bacc.Bacc("TRN2", target_bir_lowering=False, debug=False,
                   num_devices=NCORES)
    xT_d = nc.dram_tensor("xT", [128, 4, S], BF16, kind="ExternalInput")
    wqk_d = nc.dram_tensor("wqk", [128, 8, 128], BF16, kind="ExternalInput")
    wv_d = nc.dram_tensor("wv", [128, 4, 128], BF16, kind="ExternalInput")
    wo_d = nc.dram_tensor("wo", [128, 512], BF16, kind="ExternalInput")
    mk_d = nc.dram_tensor("masks", [128, NMASK, 1024], BF16, kind="ExternalInput")
    y_d = nc.dram_tensor("y", [8, 128, 4, 512], BF16, kind="ExternalOutput")

    with tile.TileContext(nc) as tc, ExitStack() as ctx:
        const = ctx.enter_context(tc.tile_pool(name="const", bufs=1))
        # SP queue: q/k weights, then x token chunks interleaved with the
        # remaining small inputs. Pool queue: masks (first half first).
        xT_t = const.tile([128, 4, S], BF16, tag="xT")
        wqk_t = const.tile([128, 8, 128], BF16, tag="wqk")
        wq_t, wk_t = wqk_t[:, 0:4, :], wqk_t[:, 4:8, :]
        nc.sync.dma_start(out=wqk_t[:, 0:4, :], in_=wqk_d[:, 0:4, :])
        nc.sync.dma_start(out=xT_t[:, :, 0:256], in_=xT_d[:, :, 0:256])
        nc.sync.dma_start(out=wqk_t[:, 4:8, :], in_=wqk_d[:, 4:8, :])
        nc.sync.dma_start(out=xT_t[:, :, 256:512], in_=xT_d[:, :, 256:512])
        mk_t = const.tile([128, NMASK, 1024], BF16, tag="mk")
        mhalf = min(4, NMASK)
        wv_t = const.tile([128, 4, 128], BF16, tag="wv")
        wo_t = const.tile([128, 512], BF16, tag="wo")
        # x over all three DMA queues; masks trail on Pool (first needed at
        # the first mask multiply, several us into the run)
        for i in (1, 3, 4, 5, 6, 7):
            eng = (nc.sync, nc.scalar, nc.gpsimd, nc.scalar,
                   nc.gpsimd, nc.sync, nc.sync)[i - 1]
            eng.dma_start(out=xT_t[:, :, i * 512:(i + 1) * 512],
                          in_=xT_d[:, :, i * 512:(i + 1) * 512])
            if i == 1:
                nc.sync.dma_start(out=wv_t[:], in_=wv_d[:, :, :])
            elif i == 3:
                nc.scalar.dma_start(out=xT_t[:, :, 1024:1536],
                                    in_=xT_d[:, :, 1024:1536])
                nc.sync.dma_start(out=wo_t[:], in_=wo_d[:, :])
        nc.gpsimd.dma_start(out=mk_t[:, 0:mhalf, :], in_=mk_d[:, 0:mhalf, :])
        if NMASK > mhalf:
            nc.gpsimd.dma_start(out=mk_t[:, mhalf:, :], in_=mk_d[:, mhalf:, :])

        qkT = const.tile([128, 2, S], BF16, tag="qkT")  # [:,0,:]=q, [:,1,:]=k
        # V: [tok_in_chunk, chunk, 130]: cols 0:64 u0-e, 64 ones, 65:129 u1-e, 129 ones
        V = const.tile([128, 32, 130], BF16, tag="V")
        nc.gpsimd.memset(V[:, :, 64:130:65], 1.0)   # only the ones columns
        identb = const.tile([128, 128], BF16, tag="identb")
        identf = const.tile([128, 128], F32, tag="identf")
        make_identity(nc, identf[:])
        nc.vector.tensor_copy(identb[:], identf[:])

        with tc.tile_pool(name="sps", bufs=2, space="PSUM") as sps, \
             tc.tile_pool(name="pvs", bufs=2, space="PSUM") as pvs, \
             tc.tile_pool(name="ops", bufs=2, space="PSUM") as ops, \
             tc.tile_pool(name="prp", bufs=14) as prp, \
             tc.tile_pool(name="aux", bufs=4) as aux, \
             tc.tile_pool(name="yp", bufs=2) as yp:

            # Lazy emitters so Python emission order matches consumption
            # order (pool slots rotate FIFO in emission order). GPSIMD/Pool
            # cannot touch PSUM, so all PSUM reads sit on DVE/Act.
            done_qk = set()     # token blocks with q/k projections emitted
            done_v = set()      # v-proj chunk pairs emitted
            probs = [{}, {}]    # unit -> chunk -> AP view [128, nq]

            def need_qk(nb):    # q+k -> qkT [dim, tok], 512-token block nb
                if nb in done_qk or "1" not in _PHASES:
                    return
                done_qk.add(nb)
                subs = ((0, 256), (256, 512)) if nb == 0 else ((0, 512),)
                for qk, w_t in ((0, wq_t), (1, wk_t)):
                    acc = ops.tile([128, 512], F32, tag="yacc")
                    for a, b in subs:
                        for dc in range(4):
                            nc.tensor.matmul(
                                acc[:, a:b], w_t[:, dc, :],
                                xT_t[:, dc, nb * 512 + a:nb * 512 + b],
                                start=(dc == 0), stop=(dc == 3))
                    if qk == 0:
                        nc.scalar.activation(
                            qkT[:, 0, nb * 512:(nb + 1) * 512], acc[:],
                            mybir.ActivationFunctionType.Copy)
                    else:
                        nc.vector.tensor_copy(
                            qkT[:, 1, nb * 512:(nb + 1) * 512], acc[:])

            def need_v(ci):     # v direct to [tok, dim] for chunks {ci, ci+1}
                ci -= ci % 2
                if ci in done_v or "1" not in _PHASES:
                    return
                done_v.add(ci)
                acc = ops.tile([128, 512], F32, tag="yacc")
                for h in range(2):
                    for dc in range(4):
                        nc.tensor.matmul(
                            acc[:, h * 128:h * 128 + 128],
                            xT_t[:, dc, (ci + h) * 128:(ci + h + 1) * 128],
                            wv_t[:, dc, :],
                            start=(dc == 0), stop=(dc == 3))
                # [128 tok, 2, 128 vdim] -> V cols {0:64, 65:129} of 2 chunks
                nc.vector.tensor_copy(
                    V[:, ci:ci + 2, :].rearrange("p c (t w) -> p c t w", t=2)[:, :, :, 0:64],
                    acc[:, 0:256].rearrange("p (c t w) -> p c t w", c=2, t=2))

            NPAIRS = len(PAIRS)

            def emit_pair(u, pi):
                p = PAIRS[pi]
                ue = slice(u * 64, u * 64 + 64)
                wdt = p["width"]
                hi_tok = (TILES[p["cs"][-1]]["hi"] + 1) * 64
                for nb in range((hi_tok + 511) // 512):
                    need_qk(nb)
                sc = sps.tile([128, 1024], F32, tag="sc")
                off = 0
                for cc in p["cs"]:
                    t = TILES[cc]
                    q0 = t["lo"] * 64
                    pos = 0
                    while pos < t["nq"]:
                        # PSUM: each matmul must stay inside one 512-col bank
                        end = min(t["nq"], pos + 512,
                                  ((off + pos) // 512 + 1) * 512 - off)
                        nc.tensor.matmul(sc[:, off + pos:off + end],
                                         qkT[ue, 1, cc * 128:(cc + 1) * 128],
                                         qkT[ue, 0, q0 + pos:q0 + end],
                                         start=True, stop=True)
                        pos = end
                    off += t["nq"]
                pr = prp.tile([128, 1024], BF16, tag="pr")
                nc.scalar.activation(pr[:, :wdt], sc[:, :wdt],
                                     mybir.ActivationFunctionType.Exp,
                                     scale=SCALE)
                nc.gpsimd.tensor_mul(pr[:, :wdt], pr[:, :wdt],
                                     mk_t[:, p["mask_id"], :wdt])
                off = 0
                for cc in p["cs"]:
                    probs[u][cc] = pr[:, off:off + TILES[cc]["nq"]]
                    off += TILES[cc]["nq"]

            def need_probs(J):  # probs + V for all chunks query-chunk J uses
                if J >= 32:
                    return
                for u in (0, 1):
                    for cc, _t, _o, _n in PVPLAN[J]:
                        if cc not in probs[u]:
                            emit_pair(u, PAIR_OF[cc])
                        need_v(cc)

            y4 = None
            state = {}          # Bk -> (pv, ao) awaiting their tail pass

            def emit_pv(Bk):    # PV + normalize for 2-J block Bk  (PE, DVE)
                # one PSUM bank per block: ao accumulation in cols 0:260
                # (J-even u0/u1, J-odd u0/u1, each 65 wide incl denominator),
                # bf16 transpose targets carved at f32 cols 320:448
                pv = pvs.tile([128, 512], F32, tag="pv")
                for J in (2 * Bk, 2 * Bk + 1):
                    base = 130 * (J % 2)
                    for u in (0, 1):
                        uv = slice(base + u * 65, base + u * 65 + 65)
                        segs = PVPLAN[J]
                        for si, (cc, toff, ooff, nr) in enumerate(segs):
                            nc.tensor.matmul(pv[ooff:ooff + nr * 64, uv],
                                             probs[u][cc][:, toff:toff + nr * 64],
                                             V[:, cc, u * 65:u * 65 + 65],
                                             start=(si == 0),
                                             stop=(si == len(segs) - 1))
                recip = aux.tile([128, 4], F32, tag="recip")
                nc.vector.reciprocal(recip[:], pv[:, 64:260:65])
                # normalize all 4 (J, unit) panels in one broadcast multiply
                ao = aux.tile([128, 4, 64], BF16, tag="ao")
                nc.vector.tensor_tensor(
                    ao[:],
                    pv[:, 0:260].rearrange("p (t w) -> p t w", t=4)[:, :, 0:64],
                    recip[:].unsqueeze(2).broadcast_to([128, 4, 64]),
                    mybir.AluOpType.mult)
                state[Bk] = (pv, ao)

            def emit_tail_a(Bk):    # transposes [PE] + aoT copy [DVE]
                pv, ao = state.pop(Bk)
                tp = pv[:, 320:448].bitcast(BF16)      # [128, 2, 128] bf16
                nc.tensor.transpose(tp[:, 0:128], ao[:, 0:2, :], identb[:])
                nc.tensor.transpose(tp[:, 128:256], ao[:, 2:4, :], identb[:])
                aoT = aux.tile([128, 256], BF16, tag="aoT")
                nc.vector.tensor_copy(aoT[:], tp)
                state[("T", Bk)] = aoT

            def emit_tail_b(Bk):    # out-proj [PE] + y copies + DMA
                nonlocal y4
                aoT = state.pop(("T", Bk))
                if Bk % 2 == 0:
                    y4 = yp.tile([128, 4, 512], BF16, tag="y4")
                for J in (2 * Bk, 2 * Bk + 1):
                    yac = ops.tile([128, 512], F32, tag="yacc")
                    nc.tensor.matmul(yac[:], aoT[:, (J % 2) * 128:(J % 2) * 128 + 128],
                                     wo_t[:], start=True, stop=True)
                    if Bk >= 12 and (Bk >= 13 or J % 2 == 0):
                        # endgame: Act is idle once the exps finish; keep DVE
                        # free for the recip/normalize/aoT chain
                        nc.scalar.activation(y4[:, J % 4, :], yac[:],
                                             mybir.ActivationFunctionType.Copy)
                    else:
                        nc.vector.tensor_copy(y4[:, J % 4, :], yac[:])
                    if Bk == 15:
                        nc.sync.dma_start(out=y_d[7][:, J % 4:J % 4 + 1, :],
                                          in_=y4[:, J % 4:J % 4 + 1, :])
                if Bk < 15:
                    half = (Bk % 2) * 2
                    nc.sync.dma_start(out=y_d[Bk // 2][:, half:half + 2, :],
                                      in_=y4[:, half:half + 2, :])

            if "2" in _PHASES:
                for J in (0, 1, 2, 3):
                    need_probs(J)
                for Bk in range(16):
                    if Bk > 0 and "3" in _PHASES:
                        emit_tail_a(Bk - 1)
                    emit_pv(Bk)
                    if Bk > 0 and "3" in _PHASES:
                        emit_tail_b(Bk - 1)
                    for J in range(2 * Bk + 4, 2 * Bk + 8):
                        need_probs(J)       # two blocks of prefetch
                if "3" in _PHASES:
                    emit_tail_a(15)
                    emit_tail_b(15)
    nc.compile()
    return nc


def _get_module():
    if "nc" not in _NC_CACHE:
        _NC_CACHE["nc"] = _build_module()
    return _NC_CACHE["nc"]


# ---------------------------------------------------------------- host

def kernel(x, w_qkv, w_out):
    x = np.asarray(x, np.float32)
    w_qkv = np.asarray(w_qkv, np.float32)
    w_out = np.asarray(w_out, np.float32)
    nc = _get_module()

    bf = ml_dtypes.bfloat16
    masks_bf = np.ascontiguousarray(MASKS.transpose(1, 0, 2)).astype(bf)
    # xT [128, 4, S]: xT[p, c, t] = x[b].reshape(S, D).T[c*128+p, t]
    xT = [np.ascontiguousarray(
            x[b].reshape(S, D).T.reshape(4, 128, S).transpose(1, 0, 2)).astype(bf)
          for b in range(B)]
    w_outT = w_out.T    # [d, e]

    def wslice(base, f):  # [128, 4, 128]: w[p, c, o] = w_qkv[base+f+o? no]
        ws = w_qkv[base + f:base + f + 128]          # [128 out, 512 in]
        return np.ascontiguousarray(
            ws.T.reshape(4, 128, 128).transpose(1, 0, 2)).astype(bf)

    in_maps = []
    for c in range(NCORES):
        b, h0 = c // 4, 2 * (c % 4)
        f = h0 * 64
        in_maps.append({
            "xT": xT[b],
            "wqk": np.concatenate([wslice(0, f), wslice(512, f)], axis=1),
            "wv": wslice(1024, f),
            "wo": np.ascontiguousarray(w_outT[f:f + 128]).astype(bf),
            "masks": masks_bf,
        })
    res = run_bass_kernel_spmd(nc, in_maps, list(range(NCORES)), trace=TRACE)
    global LAST_RESULTS
    LAST_RESULTS = res
    y = np.zeros((B, S, D), np.float32)
    for c in range(NCORES):
        yc = np.asarray(res.results[c]["y"], dtype=np.float32)  # [8,128,4,512]
        y[c // 4] += yc.transpose(0, 2, 1, 3).reshape(S, D)
    return y.reshape(B, H, W, D)


# revision 6
# speedup vs baseline: 1.0323x; 1.0069x over previous
"""Neighbourhood attention block (7x7 clamped window) on 8 Trainium2 cores, v2.

Sharding: (batch, head-pair) tensor parallel. Core c handles batch b = c//4
and heads (2*(c%4), 2*(c%4)+1). Each core computes q/k/v projections for its
two heads, neighbourhood attention, and a partial output projection in bf16;
host sums the 4 bf16 partials per batch in f32.

v2 layout: one scores tile per key chunk c (2 image rows = 128 keys), queries
= exactly the rows that see the chunk (nq = 320..704 cols), scoresT [key, q].
Probs = exp(scale*scores) * mask (bf16, one exp per chunk-pair). PV is
transposed: per query-chunk J (2 image rows = 128 queries), probs slices
[128 k, <=128 q] are the matmul stationary and V [128 k, 65] the moving
operand, accumulating ao [128 q, 130] in PSUM (cols 64/129 = softmax
denominators via ones columns in V). The reciprocal is applied on the
PSUM->SBUF copy (per-partition scalar), ao is transposed on the PE, and the
output projection consumes aoT chunks as stationary against a resident
wo [128, 512] moving operand, yielding y [128 tok, 512] per chunk.
"""
import os
import numpy as np
import ml_dtypes
from contextlib import ExitStack

_PHASES = os.environ.get("KERNEL_PHASES", "123")  # debug bisect knob

import concourse.bass as bass
import concourse.bacc as bacc
import concourse.tile as tile
import concourse.mybir as mybir
from concourse.bass_utils import run_bass_kernel_spmd
from concourse.masks import make_identity

F32 = mybir.dt.float32
BF16 = mybir.dt.bfloat16

B, H, W, D = 2, 64, 64, 512
DH, NH = 64, 8
S = H * W              # 4096 tokens per batch
KER = 7
SCALE = DH ** -0.5     # 0.125
NCORES = 8

# ---------------------------------------------------------------- geometry

def _sh(r):            # clamped window start (rows); same formula for cols
    return min(max(r - KER // 2, 0), H - KER)


def _chunks_of_row(r):  # key chunks (2 rows each) seen by query row r
    s = _sh(r)
    return list(range(s // 2, (s + KER + 1) // 2))


def _build_plan():
    # one scores tile per key chunk: queries = all rows seeing the chunk
    tiles = []          # per c: dict(c, lo, hi, nq)
    for c in range(32):
        rows = [r for r in range(H) if c in _chunks_of_row(r)]
        assert rows == list(range(rows[0], rows[-1] + 1))
        tiles.append(dict(c=c, lo=rows[0], hi=rows[-1],
                          nq=(rows[-1] - rows[0] + 1) * 64))
    assert sum(t["nq"] for t in tiles) == 64 * sum(
        len(_chunks_of_row(r)) for r in range(H))

    # exp/mask groups: greedy pairing of consecutive chunks, width <= 1024
    pairs = []          # dict(cs, width, mask_id)
    c = 0
    while c < 32:
        if c + 1 < 32 and tiles[c]["nq"] + tiles[c + 1]["nq"] <= 1024:
            pairs.append(dict(cs=[c, c + 1],
                              width=tiles[c]["nq"] + tiles[c + 1]["nq"]))
            c += 2
        else:
            pairs.append(dict(cs=[c], width=tiles[c]["nq"]))
            c += 1
    pair_of = {}        # chunk -> pair index
    for pi, p in enumerate(pairs):
        for cc in p["cs"]:
            pair_of[cc] = pi

    # PV plan per query-chunk J (rows 2J, 2J+1)
    pvplan = []
    for J in range(32):
        segs = []       # (c, tile_off_cols, out_row_off, nrows)
        for cc in sorted(set(_chunks_of_row(2 * J)) | set(_chunks_of_row(2 * J + 1))):
            rp = [r for r in (2 * J, 2 * J + 1) if cc in _chunks_of_row(r)]
            assert rp == list(range(rp[0], rp[-1] + 1))
            t = tiles[cc]
            segs.append((cc, (rp[0] - t["lo"]) * 64, (rp[0] - 2 * J) * 64,
                         len(rp)))
        # order: a full (2-row) seg opens the accumulation group, half segs
        # in the middle, and a full seg closes it (stop must cover all rows)
        full = [g for g in segs if g[3] == 2]
        half = [g for g in segs if g[3] == 1]
        assert len(full) >= 2, (J, segs)
        segs = full[:1] + half + full[1:]
        pvplan.append(segs)

    # sanity: every (query row, chunk) incidence consumed exactly once
    seen = set()
    for J, segs in enumerate(pvplan):
        for cc, toff, ooff, nr in segs:
            for k in range(nr):
                key = (2 * J + ooff // 64 + k, cc)
                assert key not in seen, key
                seen.add(key)
    for r in range(H):
        for cc in _chunks_of_row(r):
            assert (r, cc) in seen, (r, cc)

    # masks per pair (0/1), deduped. mask[k, q] over the pair's concat q-cols
    starts = np.minimum(np.maximum(np.arange(H) - KER // 2, 0), H - KER)
    valid = (np.arange(H)[None, :] >= starts[:, None]) & \
            (np.arange(H)[None, :] < starts[:, None] + KER)   # [q pos, k pos]

    def chunk_mask(cc):
        t = tiles[cc]
        ktok = cc * 128 + np.arange(128)
        qtok = t["lo"] * 64 + np.arange(t["nq"])
        return (valid[qtok[None, :] // 64, ktok[:, None] // 64]
                & valid[qtok[None, :] % 64, ktok[:, None] % 64])

    mask_list, mask_ids = [], {}
    for p in pairs:
        m = np.zeros((128, 1024), np.float32)
        off = 0
        for cc in p["cs"]:
            w = tiles[cc]["nq"]
            m[:, off:off + w] = chunk_mask(cc)
            off += w
        key = m.tobytes()
        if key not in mask_ids:
            mask_ids[key] = len(mask_list)
            mask_list.append(m)
        p["mask_id"] = mask_ids[key]
    return tiles, pairs, pair_of, pvplan, np.stack(mask_list)


TILES, PAIRS, PAIR_OF, PVPLAN, MASKS = _build_plan()
NMASK = len(MASKS)

# ---------------------------------------------------------------- device

_NC_CACHE = {}
TRACE = False          # set True (e.g. from test.py) to capture an NTFF profile
LAST_RESULTS = None    # BassKernelResults of the most recent kernel() call


def _build_module():
    nc = bacc.Bacc("TRN2", target_bir_lowering=False, debug=False,
                   num_devices=NCORES)
    xT_d = nc.dram_tensor("xT", [128, 4, S], BF16, kind="ExternalInput")
    wqk_d = nc.dram_tensor("wqk", [128, 8, 128], BF16, kind="ExternalInput")
    wv_d = nc.dram_tensor("wv", [128, 4, 128], BF16, kind="ExternalInput")
    wo_d = nc.dram_tensor("wo", [128, 512], BF16, kind="ExternalInput")
    mk_d = nc.dram_tensor("masks", [128, NMASK, 1024], BF16, kind="ExternalInput")
    y_d = nc.dram_tensor("y", [8, 128, 4, 512], BF16, kind="ExternalOutput")

    with tile.TileContext(nc) as tc, ExitStack() as ctx:
        const = ctx.enter_context(tc.tile_pool(name="const", bufs=1))
        # SP queue: q/k weights, then x token chunks interleaved with the
        # remaining small inputs. Pool queue: masks (first half first).
        xT_t = const.tile([128, 4, S], BF16, tag="xT")
        wqk_t = const.tile([128, 8, 128], BF16, tag="wqk")
        wq_t, wk_t = wqk_t[:, 0:4, :], wqk_t[:, 4:8, :]
        nc.sync.dma_start(out=wqk_t[:, 0:4, :], in_=wqk_d[:, 0:4, :])
        nc.sync.dma_start(out=xT_t[:, :, 0:256], in_=xT_d[:, :, 0:256])
        nc.sync.dma_start(out=wqk_t[:, 4:8, :], in_=wqk_d[:, 4:8, :])
        nc.sync.dma_start(out=xT_t[:, :, 256:512], in_=xT_d[:, :, 256:512])
        mk_t = const.tile([128, NMASK, 1024], BF16, tag="mk")
        mhalf = min(4, NMASK)
        wv_t = const.tile([128, 4, 128], BF16, tag="wv")
        wo_t = const.tile([128, 512], BF16, tag="wo")
        # x over all three DMA queues; masks trail on Pool (first needed at
        # the first mask multiply, several us into the run)
        for i, eng in ((1, nc.scalar), (2, nc.sync), (3, nc.sync),
                       (4, nc.scalar), (5, nc.scalar), (6, nc.sync),
                       (7, nc.sync)):
            eng.dma_start(out=xT_t[:, :, i * 512:(i + 1) * 512],
                          in_=xT_d[:, :, i * 512:(i + 1) * 512])
            if i == 2:
                nc.sync.dma_start(out=wv_t[:], in_=wv_d[:, :, :])
                nc.sync.dma_start(out=wo_t[:], in_=wo_d[:, :])
        nc.gpsimd.dma_start(out=mk_t[:, 0:mhalf, :], in_=mk_d[:, 0:mhalf, :])
        if NMASK > mhalf:
            nc.gpsimd.dma_start(out=mk_t[:, mhalf:, :], in_=mk_d[:, mhalf:, :])

        qkT = const.tile([128, 2, S], BF16, tag="qkT")  # [:,0,:]=q, [:,1,:]=k
        # V: [tok_in_chunk, chunk, 130]: cols 0:64 u0-e, 64 ones, 65:129 u1-e, 129 ones
        V = const.tile([128, 32, 130], BF16, tag="V")
        nc.gpsimd.memset(V[:, :, 64:130:65], 1.0)   # only the ones columns
        identb = const.tile([128, 128], BF16, tag="identb")
        identf = const.tile([128, 128], F32, tag="identf")
        make_identity(nc, identf[:])
        nc.vector.tensor_copy(identb[:], identf[:])

        with tc.tile_pool(name="sps", bufs=2, space="PSUM") as sps, \
             tc.tile_pool(name="pvs", bufs=2, space="PSUM") as pvs, \
             tc.tile_pool(name="ops", bufs=2, space="PSUM") as ops, \
             tc.tile_pool(name="prp", bufs=16) as prp, \
             tc.tile_pool(name="aux", bufs=8) as aux, \
             tc.tile_pool(name="yp", bufs=3) as yp:

            # Lazy emitters so Python emission order matches consumption
            # order (pool slots rotate FIFO in emission order). GPSIMD/Pool
            # cannot touch PSUM, so all PSUM reads sit on DVE/Act.
            done_qk = set()     # token blocks with q/k projections emitted
            done_v = set()      # v-proj chunk pairs emitted
            probs = [{}, {}]    # unit -> chunk -> AP view [128, nq]

            def need_qk(nb):    # q+k -> qkT [dim, tok], 512-token block nb
                if nb in done_qk or "1" not in _PHASES:
                    return
                done_qk.add(nb)
                subs = ((0, 256), (256, 512)) if nb == 0 else ((0, 512),)
                for qk, w_t in ((0, wq_t), (1, wk_t)):
                    acc = ops.tile([128, 512], F32, tag="yacc")
                    for a, b in subs:
                        for dc in range(4):
                            nc.tensor.matmul(
                                acc[:, a:b], w_t[:, dc, :],
                                xT_t[:, dc, nb * 512 + a:nb * 512 + b],
                                start=(dc == 0), stop=(dc == 3))
                    if qk == 0:
                        nc.scalar.activation(
                            qkT[:, 0, nb * 512:(nb + 1) * 512], acc[:],
                            mybir.ActivationFunctionType.Copy)
                    else:
                        nc.vector.tensor_copy(
                            qkT[:, 1, nb * 512:(nb + 1) * 512], acc[:])

            def need_v(ci):     # v direct to [tok, dim] for chunks {ci, ci+1}
                ci -= ci % 2
                if ci in done_v or "1" not in _PHASES:
                    return
                done_v.add(ci)
                acc = ops.tile([128, 512], F32, tag="yacc")
                for h in range(2):
                    for dc in range(4):
                        nc.tensor.matmul(
                            acc[:, h * 128:h * 128 + 128],
                            xT_t[:, dc, (ci + h) * 128:(ci + h + 1) * 128],
                            wv_t[:, dc, :],
                            start=(dc == 0), stop=(dc == 3))
                # [128 tok, 2, 128 vdim] -> V cols {0:64, 65:129} of 2 chunks
                nc.vector.tensor_copy(
                    V[:, ci:ci + 2, :].rearrange("p c (t w) -> p c t w", t=2)[:, :, :, 0:64],
                    acc[:, 0:256].rearrange("p (c t w) -> p c t w", c=2, t=2))

            NPAIRS = len(PAIRS)

            def emit_pair(u, pi):
                p = PAIRS[pi]
                ue = slice(u * 64, u * 64 + 64)
                wdt = p["width"]
                hi_tok = (TILES[p["cs"][-1]]["hi"] + 1) * 64
                for nb in range((hi_tok + 511) // 512):
                    need_qk(nb)
                sc = sps.tile([128, 1024], F32, tag="sc")
                off = 0
                for cc in p["cs"]:
                    t = TILES[cc]
                    q0 = t["lo"] * 64
                    pos = 0
                    while pos < t["nq"]:
                        # PSUM: each matmul must stay inside one 512-col bank
                        end = min(t["nq"], pos + 512,
                                  ((off + pos) // 512 + 1) * 512 - off)
                        nc.tensor.matmul(sc[:, off + pos:off + end],
                                         qkT[ue, 1, cc * 128:(cc + 1) * 128],
                                         qkT[ue, 0, q0 + pos:q0 + end],
                                         start=True, stop=True)
                        pos = end
                    off += t["nq"]
                pr = prp.tile([128, 1024], BF16, tag="pr")
                nc.scalar.activation(pr[:, :wdt], sc[:, :wdt],
                                     mybir.ActivationFunctionType.Exp,
                                     scale=SCALE)
                nc.gpsimd.tensor_mul(pr[:, :wdt], pr[:, :wdt],
                                     mk_t[:, p["mask_id"], :wdt])
                off = 0
                for cc in p["cs"]:
                    probs[u][cc] = pr[:, off:off + TILES[cc]["nq"]]
                    off += TILES[cc]["nq"]

            def need_probs(J):  # probs + V for all chunks query-chunk J uses
                if J >= 32:
                    return
                for u in (0, 1):
                    for cc, _t, _o, _n in PVPLAN[J]:
                        if cc not in probs[u]:
                            emit_pair(u, PAIR_OF[cc])
                        need_v(cc)

            y4 = None
            state = {}          # Bk -> (pv, ao) awaiting their tail pass

            def emit_pv(Bk):    # PV + normalize for 2-J block Bk  (PE, DVE)
                # one PSUM bank per block: ao accumulation in cols 0:260
                # (J-even u0/u1, J-odd u0/u1, each 65 wide incl denominator),
                # bf16 transpose targets carved at f32 cols 320:448
                pv = pvs.tile([128, 512], F32, tag="pv")
                for J in (2 * Bk, 2 * Bk + 1):
                    base = 130 * (J % 2)
                    for u in (0, 1):
                        uv = slice(base + u * 65, base + u * 65 + 65)
                        segs = PVPLAN[J]
                        for si, (cc, toff, ooff, nr) in enumerate(segs):
                            nc.tensor.matmul(pv[ooff:ooff + nr * 64, uv],
                                             probs[u][cc][:, toff:toff + nr * 64],
                                             V[:, cc, u * 65:u * 65 + 65],
                                             start=(si == 0),
                                             stop=(si == len(segs) - 1))
                recip = aux.tile([128, 4], F32, tag="recip")
                nc.vector.reciprocal(recip[:], pv[:, 64:260:65])
                # normalize all 4 (J, unit) panels in one broadcast multiply
                ao = aux.tile([128, 4, 64], BF16, tag="ao")
                nc.vector.tensor_tensor(
                    ao[:],
                    pv[:, 0:260].rearrange("p (t w) -> p t w", t=4)[:, :, 0:64],
                    recip[:].unsqueeze(2).broadcast_to([128, 4, 64]),
                    mybir.AluOpType.mult)
                state[Bk] = (pv, ao)

            def emit_tail_a(Bk):    # transposes [PE] + aoT copy [DVE]
                pv, ao = state.pop(Bk)
                tp = pv[:, 320:448].bitcast(BF16)      # [128, 2, 128] bf16
                nc.tensor.transpose(tp[:, 0:128], ao[:, 0:2, :], identb[:])
                nc.tensor.transpose(tp[:, 128:256], ao[:, 2:4, :], identb[:])
                aoT = aux.tile([128, 256], BF16, tag="aoT")
                nc.vector.tensor_copy(aoT[:], tp)
                state[("T", Bk)] = aoT

            def emit_tail_b(Bk):    # out-proj [PE] + y copies + DMA
                nonlocal y4
                aoT = state.pop(("T", Bk))
                if Bk % 2 == 0:
                    y4 = yp.tile([128, 4, 512], BF16, tag="y4")
                for J in (2 * Bk, 2 * Bk + 1):
                    yac = ops.tile([128, 512], F32, tag="yacc")
                    nc.tensor.matmul(yac[:], aoT[:, (J % 2) * 128:(J % 2) * 128 + 128],
                                     wo_t[:], start=True, stop=True)
                    if Bk >= 12 and (12 <= Bk < 15 or J % 2 == 0):
                        # endgame: Act is idle once the exps finish; keep DVE
                        # free for the recip/normalize/aoT chain. The very
                        # last pair splits across Act and DVE to drain faster.
                        nc.scalar.activation(y4[:, J % 4, :], yac[:],
                                             mybir.ActivationFunctionType.Copy)
                    else:
                        nc.vector.tensor_copy(y4[:, J % 4, :], yac[:])
                    if Bk == 15:
                        (nc.sync if J % 2 == 0 else nc.scalar).dma_start(
                            out=y_d[7][:, J % 4:J % 4 + 1, :],
                            in_=y4[:, J % 4:J % 4 + 1, :])
                if Bk < 15:
                    half = (Bk % 2) * 2
                    nc.sync.dma_start(out=y_d[Bk // 2][:, half:half + 2, :],
                                      in_=y4[:, half:half + 2, :])

            if "2" in _PHASES:
                for J in (0, 1, 2, 3):
                    need_probs(J)
                for Bk in range(16):
                    if Bk > 0 and "3" in _PHASES:
                        emit_tail_a(Bk - 1)
                    emit_pv(Bk)
                    if Bk > 0 and "3" in _PHASES:
                        emit_tail_b(Bk - 1)
                    for J in range(2 * Bk + 4, 2 * Bk + 8):
                        need_probs(J)       # two blocks of prefetch
                if "3" in _PHASES:
                    emit_tail_a(15)
                    emit_tail_b(15)
    nc.compile()
    return nc


def _get_module():
    if "nc" not in _NC_CACHE:
        _NC_CACHE["nc"] = _build_module()
    return _NC_CACHE["nc"]


# ---------------------------------------------------------------- host

def kernel(x, w_qkv, w_out):
    x = np.asarray(x, np.float32)
    w_qkv = np.asarray(w_qkv, np.float32)
    w_out = np.asarray(w_out, np.float32)
    nc = _get_module()

    bf = ml_dtypes.bfloat16
    masks_bf = np.ascontiguousarray(MASKS.transpose(1, 0, 2)).astype(bf)
    # xT [128, 4, S]: xT[p, c, t] = x[b].reshape(S, D).T[c*128+p, t]
    xT = [np.ascontiguousarray(
            x[b].reshape(S, D).T.reshape(4, 128, S).transpose(1, 0, 2)).astype(bf)
          for b in range(B)]
    w_outT = w_out.T    # [d, e]

    def wslice(base, f):  # [128, 4, 128]: w[p, c, o] = w_qkv[base+f+o? no]
        ws = w_qkv[base + f:base + f + 128]          # [128 out, 512 in]
        return np.ascontiguousarray(
            ws.T.reshape(4, 128, 128).transpose(1, 0, 2)).astype(bf)

    in_maps = []
    for c in range(NCORES):
        b, h0 = c // 4, 2 * (c % 4)
        f = h0 * 64
        in_maps.append({
            "xT": xT[b],
            "wqk": np.concatenate([wslice(0, f), wslice(512, f)], axis=1),
            "wv": wslice(1024, f),
            "wo": np.ascontiguousarray(w_outT[f:f + 128]).astype(bf),
            "masks": masks_bf,
        })
    res = run_bass_kernel_spmd(nc, in_maps, list(range(NCORES)), trace=TRACE)
    global LAST_RESULTS
    LAST_RESULTS = res
    y = np.zeros((B, S, D), np.float32)
    for c in range(NCORES):
        yc = np.asarray(res.results[c]["y"], dtype=np.float32)  # [8,128,4,512]
        y[c // 4] += yc.transpose(0, 2, 1, 3).reshape(S, D)
    return y.reshape(B, H, W, D)
